# revision 2
# baseline (speedup 1.0000x reference)
"""Trainium2 Bass kernel for the LogicLayer (gnn_message_passing) problem.

out[n, y] = k0[y] + k1[y]*a + k2[y]*b + k3[y]*(a*b)
  with a = x[n, a_idx[y]], b = x[n, b_idx[y]],
  k = softmax(weights, -1) @ GATE_COEFFS          (per output neuron y)

Strategy (8 NeuronCores, data-parallel over batch — 512 rows/core):
  1. On-device softmax of weights -> 4 coefficient vectors, transposed on
     TensorE into "gathered" layout kg[j][q, g] = k_j(y = g*128 + q).
  2. Phase 1: transpose the core's x shard [512, 16384] into xT [16384, 512]
     in DRAM via TensorE 128x128 transposes (gathers need feature-major rows).
  3. Phase 2: per 1024-output chunk, two HW dma_gather ops pull the a/b rows
     (2KB contiguous each) into SBUF with outputs laid out [128, 8, 512]
     (y%128 on partitions). ACT computes u=k1*A+k0, v=k3*A+k2 (per-partition
     scale/bias), DVE computes o=u+v*B, TensorE transposes back to
     batch-major, and contiguous DMAs write the output block.
"""

import numpy as np

_GATE_COEFFS = np.array(
    [
        [0.0, 0.0, 0.0, 0.0],
        [0.0, 0.0, 0.0, 1.0],
        [0.0, 1.0, 0.0, -1.0],
        [0.0, 1.0, 0.0, 0.0],
        [0.0, 0.0, 1.0, -1.0],
        [0.0, 0.0, 1.0, 0.0],
        [0.0, 1.0, 1.0, -2.0],
        [0.0, 1.0, 1.0, -1.0],
        [1.0, -1.0, -1.0, 1.0],
        [1.0, -1.0, -1.0, 2.0],
        [1.0, 0.0, -1.0, 0.0],
        [1.0, 0.0, -1.0, 1.0],
        [1.0, -1.0, 0.0, 0.0],
        [1.0, -1.0, 0.0, 1.0],
        [1.0, 0.0, 0.0, -1.0],
        [1.0, 0.0, 0.0, 0.0],
    ],
    dtype=np.float32,
)

BATCH, IN_DIM, OUT_DIM = 4096, 16384, 16384
NCORES = 8
B = BATCH // NCORES  # 512 batch rows per core
YC = 1024            # outputs per gather chunk
G = YC // 128        # 8 groups of 128 outputs per chunk
NCHUNK = OUT_DIM // YC
FC = 2048            # feature columns per phase-1 load
NFC = IN_DIM // FC
NB = B // 128        # 4 batch tiles per core

_PROGRAM_CACHE = {}


def _wrap_idx(idx: np.ndarray) -> np.ndarray:
    """Pack per-chunk gather indices in the SWDGE wrapped-int16 layout.

    Within chunk t, local index i lives at [i % 16, t*(YC//16) + i//16],
    replicated across all eight 16-partition groups.
    """
    a = idx.astype(np.int16).reshape(NCHUNK, YC // 16, 16)
    w = np.ascontiguousarray(a.transpose(2, 0, 1)).reshape(16, NCHUNK * (YC // 16))
    return np.ascontiguousarray(np.tile(w, (8, 1)))


def _build_program():
    import concourse.bass as bass  # noqa: F401
    import concourse.tile as tile
    from concourse import bacc, mybir

    f32 = mybir.dt.float32
    i16 = mybir.dt.int16
    AF = mybir.ActivationFunctionType
    ALU = mybir.AluOpType

    nc = bacc.Bacc("TRN2", target_bir_lowering=False, debug=False)
    x_h = nc.dram_tensor("x", [B, IN_DIM], f32, kind="ExternalInput")
    w_h = nc.dram_tensor("w16", [OUT_DIM, 16], f32, kind="ExternalInput")
    ia_h = nc.dram_tensor("ia", [128, OUT_DIM // 16], i16, kind="ExternalInput")
    ib_h = nc.dram_tensor("ib", [128, OUT_DIM // 16], i16, kind="ExternalInput")
    gm_h = nc.dram_tensor("gm", [4, 128, 2048], f32, kind="ExternalInput")
    id_h = nc.dram_tensor("ident", [128, 128], f32, kind="ExternalInput")
    out_h = nc.dram_tensor("out", [B, OUT_DIM], f32, kind="ExternalOutput")

    with tile.TileContext(nc) as tc:
        from contextlib import ExitStack

        with ExitStack() as stack:
            cp = stack.enter_context(tc.tile_pool(name="const", bufs=1))
            dram = stack.enter_context(tc.tile_pool(name="dram", bufs=1, space="DRAM"))

            ident = cp.tile([128, 128], f32)
            nc.sync.dma_start(ident[:], id_h.ap()[:, :])
            ia_sb = cp.tile([128, OUT_DIM // 16], i16)
            nc.sync.dma_start(ia_sb[:], ia_h.ap()[:, :])
            ib_sb = cp.tile([128, OUT_DIM // 16], i16)
            nc.sync.dma_start(ib_sb[:], ib_h.ap()[:, :])
            kg = [
                cp.tile([128, 128], f32, tag=f"kg{j}", name=f"kg{j}")
                for j in range(4)
            ]

            xT = dram.tile([IN_DIM, B], f32)

            # ---- coefficients: k = softmax(weights) @ GATE_COEFFS ----
            with (
                tc.tile_pool(name="kcalc", bufs=1) as kp,
                tc.tile_pool(name="kpsum", bufs=2, space="PSUM") as kps,
            ):
                w_sb = kp.tile([128, 2048], f32, tag="wsb")
                nc.sync.dma_start(
                    w_sb[:], w_h.ap().rearrange("(p r) g -> p (r g)", p=128)
                )
                e_sb = kp.tile([128, 2048], f32, tag="esb")
                nc.scalar.activation(e_sb[:], w_sb[:], AF.Exp)
                s_sb = kp.tile([128, 128], f32, tag="ssb")
                nc.vector.tensor_reduce(
                    s_sb[:],
                    e_sb[:].rearrange("p (r g) -> p r g", g=16),
                    mybir.AxisListType.X,
                    ALU.add,
                )
                r_sb = kp.tile([128, 128], f32, tag="rsb")
                nc.vector.reciprocal(r_sb[:], s_sb[:])
                for j in range(4):
                    gm_sb = kp.tile([128, 2048], f32, tag="gmsb")
                    nc.sync.dma_start(gm_sb[:], gm_h.ap()[j])
                    t1 = kp.tile([128, 2048], f32, tag="t1")
                    nc.vector.tensor_mul(t1[:], e_sb[:], gm_sb[:])
                    kraw = kp.tile([128, 128], f32, tag="kraw")
                    nc.vector.tensor_reduce(
                        kraw[:],
                        t1[:].rearrange("p (r g) -> p r g", g=16),
                        mybir.AxisListType.X,
                        ALU.add,
                    )
                    kj = kp.tile([128, 128], f32, tag="kj")
                    nc.vector.tensor_mul(kj[:], kraw[:], r_sb[:])
                    pk = kps.tile([128, 128], f32)
                    nc.tensor.transpose(pk[:], kj[:], ident[:])
                    nc.scalar.copy(kg[j][:], pk[:])

            # ---- phase 1: x [512, 16384] -> xT [16384, 512] in DRAM ----
            with (
                tc.tile_pool(name="ph1", bufs=2) as p1,
                tc.tile_pool(name="ph1o", bufs=4) as p1o,
                tc.tile_pool(name="ph1ps", bufs=4, space="PSUM") as p1ps,
            ):
                for fc in range(NFC):
                    xin = []
                    for nb in range(NB):
                        t = p1.tile([128, FC], f32, tag=f"xin{nb}")
                        nc.sync.dma_start(
                            t[:],
                            x_h.ap()[
                                nb * 128 : (nb + 1) * 128, fc * FC : (fc + 1) * FC
                            ],
                        )
                        xin.append(t)
                    for fbl in range(FC // 128):
                        ps = p1ps.tile([128, 512], f32)
                        for nb in range(NB):
                            nc.tensor.transpose(
                                ps[:, nb * 128 : (nb + 1) * 128],
                                xin[nb][:, fbl * 128 : (fbl + 1) * 128],
                                ident[:],
                            )
                        xts = p1o.tile([128, 512], f32)
                        nc.scalar.copy(xts[:], ps[:])
                        row0 = fc * FC + fbl * 128
                        nc.sync.dma_start(xT[row0 : row0 + 128, :], xts[:])

            # ---- phase 2: gather + multilinear + transpose-back + store ----
            with (
                tc.tile_pool(name="p2", bufs=2) as p2,
                tc.tile_pool(name="p2o", bufs=2) as p2o,
                tc.tile_pool(name="p2ps", bufs=4, space="PSUM") as p2ps,
            ):
                SC = YC // 16  # idx columns per chunk
                for t in range(NCHUNK):
                    A = p2.tile([128, G, 512], f32, tag="A")
                    nc.gpsimd.dma_gather(
                        out_ap=A[:],
                        in_ap=xT[:, :],
                        idxs_ap=ia_sb[:, t * SC : (t + 1) * SC],
                        num_idxs=YC,
                        num_idxs_reg=YC,
                        elem_size=B,
                    )
                    Bt = p2.tile([128, G, 512], f32, tag="B")
                    nc.gpsimd.dma_gather(
                        out_ap=Bt[:],
                        in_ap=xT[:, :],
                        idxs_ap=ib_sb[:, t * SC : (t + 1) * SC],
                        num_idxs=YC,
                        num_idxs_reg=YC,
                        elem_size=B,
                    )
                    u = p2.tile([128, G, 512], f32, tag="u")
                    v = p2.tile([128, G, 512], f32, tag="v")
                    for gl in range(G):
                        gi = t * G + gl
                        nc.scalar.activation(
                            u[:, gl, :],
                            A[:, gl, :],
                            AF.Identity,
                            bias=kg[0][:, gi : gi + 1],
                            scale=kg[1][:, gi : gi + 1],
                        )
                        nc.scalar.activation(
                            v[:, gl, :],
                            A[:, gl, :],
                            AF.Identity,
                            bias=kg[2][:, gi : gi + 1],
                            scale=kg[3][:, gi : gi + 1],
                        )
                    nc.vector.tensor_mul(v[:], v[:], Bt[:])
                    nc.vector.tensor_add(u[:], u[:], v[:])
                    for nb in range(NB):
                        osb = p2o.tile([128, YC], f32, tag=f"osb{nb}")
                        for gh in range(G // 4):
                            ps = p2ps.tile([128, 512], f32)
                            for gq in range(4):
                                gl = gh * 4 + gq
                                nc.tensor.transpose(
                                    ps[:, gq * 128 : (gq + 1) * 128],
                                    u[:, gl, nb * 128 : (nb + 1) * 128],
                                    ident[:],
                                )
                            nc.vector.tensor_copy(
                                osb[:, gh * 512 : (gh + 1) * 512], ps[:]
                            )
                        nc.sync.dma_start(
                            out_h.ap()[
                                nb * 128 : (nb + 1) * 128, t * YC : (t + 1) * YC
                            ],
                            osb[:],
                        )

    nc.compile()
    return nc


def kernel(x, weights, a_idx, b_idx):
    from concourse.bass_utils import run_bass_kernel_spmd

    x = np.asarray(x, dtype=np.float32)
    weights = np.asarray(weights, dtype=np.float32)
    a_idx = np.asarray(a_idx)
    b_idx = np.asarray(b_idx)

    if "nc" not in _PROGRAM_CACHE:
        _PROGRAM_CACHE["nc"] = _build_program()
    nc = _PROGRAM_CACHE["nc"]

    ia = _wrap_idx(a_idx)
    ib = _wrap_idx(b_idx)
    gm = np.ascontiguousarray(
        np.broadcast_to(
            np.tile(_GATE_COEFFS.T, (1, 128))[:, None, :], (4, 128, 2048)
        )
    ).astype(np.float32)
    ident = np.eye(128, dtype=np.float32)

    in_maps = []
    for c in range(NCORES):
        in_maps.append(
            {
                "x": np.ascontiguousarray(x[c * B : (c + 1) * B]),
                "w16": weights,
                "ia": ia,
                "ib": ib,
                "gm": gm,
                "ident": ident,
            }
        )

    res = run_bass_kernel_spmd(nc, in_maps, list(range(NCORES)))
    out = np.concatenate([res.results[c]["out"] for c in range(NCORES)], axis=0)
    return out


# revision 5
# speedup vs baseline: 1.0419x; 1.0419x over previous
"""Trainium2 Bass kernel for the LogicLayer (gnn_message_passing) problem.

out[n, y] = k0[y] + k1[y]*a + k2[y]*b + k3[y]*(a*b)
  with a = x[n, a_idx[y]], b = x[n, b_idx[y]],
  k = softmax(weights, -1) @ GATE_COEFFS          (per output neuron y)

Strategy (8 NeuronCores, data-parallel over batch — 512 rows/core):
  1. On-device softmax of weights -> 4 coefficient vectors, transposed on
     TensorE into "gathered" layout kg[j][q, g] = k_j(y = g*128 + q).
  2. Phase 1: transpose the core's x shard [512, 16384] into xT [16384, 512]
     in DRAM via TensorE 128x128 transposes (gathers need feature-major rows).
  3. Phase 2: per 1024-output chunk, two HW dma_gather ops pull the a/b rows
     (2KB contiguous each) into SBUF with outputs laid out [128, 8, 512]
     (y%128 on partitions). ACT computes u=k1*A+k0, v=k3*A+k2 (per-partition
     scale/bias), DVE computes o=u+v*B, TensorE transposes back to
     batch-major, and contiguous DMAs write the output block.
"""

import numpy as np

_GATE_COEFFS = np.array(
    [
        [0.0, 0.0, 0.0, 0.0],
        [0.0, 0.0, 0.0, 1.0],
        [0.0, 1.0, 0.0, -1.0],
        [0.0, 1.0, 0.0, 0.0],
        [0.0, 0.0, 1.0, -1.0],
        [0.0, 0.0, 1.0, 0.0],
        [0.0, 1.0, 1.0, -2.0],
        [0.0, 1.0, 1.0, -1.0],
        [1.0, -1.0, -1.0, 1.0],
        [1.0, -1.0, -1.0, 2.0],
        [1.0, 0.0, -1.0, 0.0],
        [1.0, 0.0, -1.0, 1.0],
        [1.0, -1.0, 0.0, 0.0],
        [1.0, -1.0, 0.0, 1.0],
        [1.0, 0.0, 0.0, -1.0],
        [1.0, 0.0, 0.0, 0.0],
    ],
    dtype=np.float32,
)

BATCH, IN_DIM, OUT_DIM = 4096, 16384, 16384
NCORES = 8
B = BATCH // NCORES  # 512 batch rows per core
YC = 1024            # outputs per gather chunk
G = YC // 128        # 8 groups of 128 outputs per chunk
NCHUNK = OUT_DIM // YC
FC = 2048            # feature columns per phase-1 load
NFC = IN_DIM // FC
NB = B // 128        # 4 batch tiles per core

_PROGRAM_CACHE = {}


def _wrap_idx(idx: np.ndarray) -> np.ndarray:
    """Pack per-chunk gather indices in the SWDGE wrapped-int16 layout.

    Within chunk t, local index i lives at [i % 16, t*(YC//16) + i//16],
    replicated across all eight 16-partition groups.
    """
    a = idx.astype(np.int16).reshape(NCHUNK, YC // 16, 16)
    w = np.ascontiguousarray(a.transpose(2, 0, 1)).reshape(16, NCHUNK * (YC // 16))
    return np.ascontiguousarray(np.tile(w, (8, 1)))


def _build_program():
    import concourse.bass as bass  # noqa: F401
    import concourse.tile as tile
    from concourse import bacc, mybir

    f32 = mybir.dt.float32
    i16 = mybir.dt.int16
    AF = mybir.ActivationFunctionType
    ALU = mybir.AluOpType

    nc = bacc.Bacc("TRN2", target_bir_lowering=False, debug=False)
    x_h = nc.dram_tensor("x", [B, IN_DIM], f32, kind="ExternalInput")
    w_h = nc.dram_tensor("w16", [OUT_DIM, 16], f32, kind="ExternalInput")
    ia_h = nc.dram_tensor("ia", [128, OUT_DIM // 16], i16, kind="ExternalInput")
    ib_h = nc.dram_tensor("ib", [128, OUT_DIM // 16], i16, kind="ExternalInput")
    gm_h = nc.dram_tensor("gm", [4, 128, 2048], f32, kind="ExternalInput")
    id_h = nc.dram_tensor("ident", [128, 128], f32, kind="ExternalInput")
    out_h = nc.dram_tensor("out", [B, OUT_DIM], f32, kind="ExternalOutput")

    with tile.TileContext(nc) as tc:
        from contextlib import ExitStack

        with ExitStack() as stack:
            cp = stack.enter_context(tc.tile_pool(name="const", bufs=1))
            dram = stack.enter_context(tc.tile_pool(name="dram", bufs=1, space="DRAM"))

            ident = cp.tile([128, 128], f32)
            nc.sync.dma_start(ident[:], id_h.ap()[:, :])
            ia_sb = cp.tile([128, OUT_DIM // 16], i16)
            nc.sync.dma_start(ia_sb[:], ia_h.ap()[:, :])
            ib_sb = cp.tile([128, OUT_DIM // 16], i16)
            nc.sync.dma_start(ib_sb[:], ib_h.ap()[:, :])
            kg = [
                cp.tile([128, 128], f32, tag=f"kg{j}", name=f"kg{j}")
                for j in range(4)
            ]

            xT = dram.tile([IN_DIM, B], f32)

            # ---- coefficients: k = softmax(weights) @ GATE_COEFFS ----
            with (
                tc.tile_pool(name="kcalc", bufs=1) as kp,
                tc.tile_pool(name="kpsum", bufs=2, space="PSUM") as kps,
            ):
                w_sb = kp.tile([128, 2048], f32, tag="wsb")
                nc.sync.dma_start(
                    w_sb[:], w_h.ap().rearrange("(p r) g -> p (r g)", p=128)
                )
                e_sb = kp.tile([128, 2048], f32, tag="esb")
                nc.scalar.activation(e_sb[:], w_sb[:], AF.Exp)
                s_sb = kp.tile([128, 128], f32, tag="ssb")
                nc.vector.tensor_reduce(
                    s_sb[:],
                    e_sb[:].rearrange("p (r g) -> p r g", g=16),
                    mybir.AxisListType.X,
                    ALU.add,
                )
                r_sb = kp.tile([128, 128], f32, tag="rsb")
                nc.vector.reciprocal(r_sb[:], s_sb[:])
                for j in range(4):
                    gm_sb = kp.tile([128, 2048], f32, tag="gmsb")
                    nc.sync.dma_start(gm_sb[:], gm_h.ap()[j])
                    t1 = kp.tile([128, 2048], f32, tag="t1")
                    nc.vector.tensor_mul(t1[:], e_sb[:], gm_sb[:])
                    kraw = kp.tile([128, 128], f32, tag="kraw")
                    nc.vector.tensor_reduce(
                        kraw[:],
                        t1[:].rearrange("p (r g) -> p r g", g=16),
                        mybir.AxisListType.X,
                        ALU.add,
                    )
                    kj = kp.tile([128, 128], f32, tag="kj")
                    nc.vector.tensor_mul(kj[:], kraw[:], r_sb[:])
                    pk = kps.tile([128, 128], f32)
                    nc.tensor.transpose(pk[:], kj[:], ident[:])
                    nc.scalar.copy(kg[j][:], pk[:])

            # ---- phase 1: x [512, 16384] -> xT [16384, 512] in DRAM ----
            # x loads go on the sync queue, xT writes on the vector queue so
            # prefetch of the next column block is never stuck behind store
            # descriptors in the same HWDGE FIFO.
            with (
                tc.tile_pool(name="ph1", bufs=3) as p1,
                tc.tile_pool(name="ph1o", bufs=8) as p1o,
                tc.tile_pool(name="ph1ps", bufs=6, space="PSUM") as p1ps,
            ):
                for fc in range(NFC):
                    xin = []
                    for nb in range(NB):
                        t = p1.tile([128, FC], f32, tag=f"xin{nb}")
                        nc.sync.dma_start(
                            t[:],
                            x_h.ap()[
                                nb * 128 : (nb + 1) * 128, fc * FC : (fc + 1) * FC
                            ],
                        )
                        xin.append(t)
                    for fbl in range(FC // 128):
                        ps = p1ps.tile([128, 512], f32)
                        for nb in range(NB):
                            nc.tensor.transpose(
                                ps[:, nb * 128 : (nb + 1) * 128],
                                xin[nb][:, fbl * 128 : (fbl + 1) * 128],
                                ident[:],
                            )
                        xts = p1o.tile([128, 512], f32)
                        nc.scalar.copy(xts[:], ps[:])
                        row0 = fc * FC + fbl * 128
                        nc.gpsimd.dma_start(xT[row0 : row0 + 128, :], xts[:])

            # ---- phase 2: gather + multilinear + transpose-back + store ----
            with (
                tc.tile_pool(name="p2", bufs=2) as p2,
                tc.tile_pool(name="p2o", bufs=2) as p2o,
                tc.tile_pool(name="p2ps", bufs=4, space="PSUM") as p2ps,
            ):
                SC = YC // 16  # idx columns per chunk
                for t in range(NCHUNK):
                    A = p2.tile([128, G, 512], f32, tag="A")
                    nc.gpsimd.dma_gather(
                        out_ap=A[:],
                        in_ap=xT[:, :],
                        idxs_ap=ia_sb[:, t * SC : (t + 1) * SC],
                        num_idxs=YC,
                        num_idxs_reg=YC,
                        elem_size=B,
                    )
                    Bt = p2.tile([128, G, 512], f32, tag="B")
                    nc.gpsimd.dma_gather(
                        out_ap=Bt[:],
                        in_ap=xT[:, :],
                        idxs_ap=ib_sb[:, t * SC : (t + 1) * SC],
                        num_idxs=YC,
                        num_idxs_reg=YC,
                        elem_size=B,
                    )
                    u = p2.tile([128, G, 512], f32, tag="u")
                    v = p2.tile([128, G, 512], f32, tag="v")
                    for gl in range(G):
                        gi = t * G + gl
                        nc.scalar.activation(
                            u[:, gl, :],
                            A[:, gl, :],
                            AF.Identity,
                            bias=kg[0][:, gi : gi + 1],
                            scale=kg[1][:, gi : gi + 1],
                        )
                        nc.scalar.activation(
                            v[:, gl, :],
                            A[:, gl, :],
                            AF.Identity,
                            bias=kg[2][:, gi : gi + 1],
                            scale=kg[3][:, gi : gi + 1],
                        )
                    nc.vector.tensor_mul(v[:], v[:], Bt[:])
                    # out = transpose(u) + transpose(v*B), accumulated in PSUM
                    # by back-to-back transpose matmuls (start/stop group).
                    for nb in range(NB):
                        osb = p2o.tile([128, YC], f32, tag=f"osb{nb}")
                        for gh in range(G // 4):
                            ps = p2ps.tile([128, 512], f32)
                            for gq in range(4):
                                gl = gh * 4 + gq
                                nc.tensor.matmul(
                                    ps[:, gq * 128 : (gq + 1) * 128],
                                    u[:, gl, nb * 128 : (nb + 1) * 128],
                                    ident[:],
                                    is_transpose=True,
                                    start=True,
                                    stop=False,
                                )
                                nc.tensor.matmul(
                                    ps[:, gq * 128 : (gq + 1) * 128],
                                    v[:, gl, nb * 128 : (nb + 1) * 128],
                                    ident[:],
                                    is_transpose=True,
                                    start=False,
                                    stop=True,
                                )
                            nc.vector.tensor_copy(
                                osb[:, gh * 512 : (gh + 1) * 512], ps[:]
                            )
                        nc.sync.dma_start(
                            out_h.ap()[
                                nb * 128 : (nb + 1) * 128, t * YC : (t + 1) * YC
                            ],
                            osb[:],
                        )

    nc.compile()
    return nc


def kernel(x, weights, a_idx, b_idx):
    from concourse.bass_utils import run_bass_kernel_spmd

    x = np.asarray(x, dtype=np.float32)
    weights = np.asarray(weights, dtype=np.float32)
    a_idx = np.asarray(a_idx)
    b_idx = np.asarray(b_idx)

    if "nc" not in _PROGRAM_CACHE:
        _PROGRAM_CACHE["nc"] = _build_program()
    nc = _PROGRAM_CACHE["nc"]

    ia = _wrap_idx(a_idx)
    ib = _wrap_idx(b_idx)
    gm = np.ascontiguousarray(
        np.broadcast_to(
            np.tile(_GATE_COEFFS.T, (1, 128))[:, None, :], (4, 128, 2048)
        )
    ).astype(np.float32)
    ident = np.eye(128, dtype=np.float32)

    in_maps = []
    for c in range(NCORES):
        in_maps.append(
            {
                "x": np.ascontiguousarray(x[c * B : (c + 1) * B]),
                "w16": weights,
                "ia": ia,
                "ib": ib,
                "gm": gm,
                "ident": ident,
            }
        )

    res = run_bass_kernel_spmd(nc, in_maps, list(range(NCORES)))
    out = np.concatenate([res.results[c]["out"] for c in range(NCORES)], axis=0)
    return out


# revision 7
# speedup vs baseline: 1.5663x; 1.5033x over previous
"""Trainium2 Bass kernel for the LogicLayer (gnn_message_passing) problem.

out[n, y] = k0[y] + k1[y]*a + k2[y]*b + k3[y]*(a*b)
  with a = x[n, a_idx[y]], b = x[n, b_idx[y]],
  k = softmax(weights, -1) @ GATE_COEFFS          (per output neuron y)

Strategy (8 NeuronCores, data-parallel over batch — 512 rows/core):
  * The core's x shard is uploaded pre-transposed (xT [16384, 512], feature
    -major) so the on-device gathers are contiguous 2KB-row DMA reads.
  * On-device softmax of weights -> 4 coefficient vectors; TensorE transpose
    puts them in gathered layout kg[j][q, g] = k_j(y = g*128 + q).
  * Per 1024-output chunk: two indexed-DMA gathers pull a/b rows into SBUF
    laid out [128, 8, 512] (y%128 on partitions, 8 y-groups, 512 batch).
    ACT computes u=k1*A+k0 and v=k3*A+k2 with per-partition scale/bias,
    DVE computes v*=B, TensorE transposes u and v back to batch-major while
    accumulating u^T+v^T in PSUM, and contiguous DMAs store the result.
"""

import numpy as np

_GATE_COEFFS = np.array(
    [
        [0.0, 0.0, 0.0, 0.0],
        [0.0, 0.0, 0.0, 1.0],
        [0.0, 1.0, 0.0, -1.0],
        [0.0, 1.0, 0.0, 0.0],
        [0.0, 0.0, 1.0, -1.0],
        [0.0, 0.0, 1.0, 0.0],
        [0.0, 1.0, 1.0, -2.0],
        [0.0, 1.0, 1.0, -1.0],
        [1.0, -1.0, -1.0, 1.0],
        [1.0, -1.0, -1.0, 2.0],
        [1.0, 0.0, -1.0, 0.0],
        [1.0, 0.0, -1.0, 1.0],
        [1.0, -1.0, 0.0, 0.0],
        [1.0, -1.0, 0.0, 1.0],
        [1.0, 0.0, 0.0, -1.0],
        [1.0, 0.0, 0.0, 0.0],
    ],
    dtype=np.float32,
)

BATCH, IN_DIM, OUT_DIM = 4096, 16384, 16384
NCORES = 8
B = BATCH // NCORES  # 512 batch rows per core
YC = 1024            # outputs per gather chunk
G = YC // 128        # 8 groups of 128 outputs per chunk
NCHUNK = OUT_DIM // YC
NB = B // 128        # 4 batch tiles per core

# "indirect": GPSIMD indirect_dma_start (dynamic-AP DMA)
# "swdge":    GPSIMD dma_gather (Q7 ucode descriptor generation)
GATHER_MODE = "swdge"

_PROGRAM_CACHE = {}


def _wrap_idx_swdge(idx: np.ndarray) -> np.ndarray:
    """dma_gather layout: within chunk t, item i -> y = base + i; idx i lives
    at [i % 16, t*(YC//16) + i//16], replicated across the 8 16-part groups."""
    a = idx.astype(np.int16).reshape(NCHUNK, G, 128)  # [t, g, p]: i = g*128+p
    a = a.transpose(0, 2, 1)  # -> [t, p, g] so i order is restored below
    # rebuild flat order i = g*128 + p per chunk
    flat = idx.astype(np.int16).reshape(NCHUNK, YC)
    # we want item i of chunk t to be y-target base + 128*(i%... see kernel:
    # position i corresponds to y = base + (i % 128) ... dma_gather maps item
    # i -> partition i%128, slot i//128, i.e. y = base + (i//128)*0 ... the
    # kernel uses y = base + g*128 + q with q = i % 128, g = i // 128, so the
    # natural order flat[t, i] = a_idx[base + i] is exactly right.
    w = flat.reshape(NCHUNK, YC // 16, 16)  # [t, s, p16]
    w = np.ascontiguousarray(w.transpose(2, 0, 1)).reshape(16, NCHUNK * (YC // 16))
    return np.ascontiguousarray(np.tile(w, (8, 1)))


def _wrap_idx_indirect(idx: np.ndarray) -> np.ndarray:
    """indirect_dma_start layout: offsets iterate in AP order k = p*G + g over
    the [128, G] slice; item k writes out run (p=k//G, g=k%G). The kernel wants
    A[p, g, :] to be the value for y = base + 128*g + p."""
    a = idx.astype(np.int32).reshape(NCHUNK, G, 128)  # [t, g, p]
    return np.ascontiguousarray(a.transpose(2, 0, 1).reshape(128, NCHUNK * G))


def _build_program():
    import concourse.bass as bass
    import concourse.tile as tile
    from concourse import bacc, mybir

    f32 = mybir.dt.float32
    AF = mybir.ActivationFunctionType
    ALU = mybir.AluOpType

    nc = bacc.Bacc("TRN2", target_bir_lowering=False, debug=False)
    xT_h = nc.dram_tensor("xT", [IN_DIM, B], f32, kind="ExternalInput")
    w_h = nc.dram_tensor("w16", [OUT_DIM, 16], f32, kind="ExternalInput")
    if GATHER_MODE == "indirect":
        idt = mybir.dt.int32
        ia_h = nc.dram_tensor("ia", [128, NCHUNK * G], idt, kind="ExternalInput")
        ib_h = nc.dram_tensor("ib", [128, NCHUNK * G], idt, kind="ExternalInput")
    else:
        idt = mybir.dt.int16
        ia_h = nc.dram_tensor("ia", [128, OUT_DIM // 16], idt, kind="ExternalInput")
        ib_h = nc.dram_tensor("ib", [128, OUT_DIM // 16], idt, kind="ExternalInput")
    gm_h = nc.dram_tensor("gm", [4, 128, 2048], f32, kind="ExternalInput")
    id_h = nc.dram_tensor("ident", [128, 128], f32, kind="ExternalInput")
    out_h = nc.dram_tensor("out", [B, OUT_DIM], f32, kind="ExternalOutput")

    with tile.TileContext(nc) as tc:
        from contextlib import ExitStack

        with ExitStack() as stack:
            cp = stack.enter_context(tc.tile_pool(name="const", bufs=1))

            ident = cp.tile([128, 128], f32)
            nc.sync.dma_start(ident[:], id_h.ap()[:, :])
            ia_sb = cp.tile(list(ia_h.shape), idt)
            nc.sync.dma_start(ia_sb[:], ia_h.ap()[:, :])
            ib_sb = cp.tile(list(ib_h.shape), idt)
            nc.sync.dma_start(ib_sb[:], ib_h.ap()[:, :])
            kg = [
                cp.tile([128, 128], f32, tag=f"kg{j}", name=f"kg{j}")
                for j in range(4)
            ]

            # ---- coefficients: k = softmax(weights) @ GATE_COEFFS ----
            with (
                tc.tile_pool(name="kcalc", bufs=1) as kp,
                tc.tile_pool(name="kpsum", bufs=2, space="PSUM") as kps,
            ):
                w_sb = kp.tile([128, 2048], f32, tag="wsb")
                nc.sync.dma_start(
                    w_sb[:], w_h.ap().rearrange("(p r) g -> p (r g)", p=128)
                )
                e_sb = kp.tile([128, 2048], f32, tag="esb")
                nc.scalar.activation(e_sb[:], w_sb[:], AF.Exp)
                s_sb = kp.tile([128, 128], f32, tag="ssb")
                nc.vector.tensor_reduce(
                    s_sb[:],
                    e_sb[:].rearrange("p (r g) -> p r g", g=16),
                    mybir.AxisListType.X,
                    ALU.add,
                )
                r_sb = kp.tile([128, 128], f32, tag="rsb")
                nc.vector.reciprocal(r_sb[:], s_sb[:])
                for j in range(4):
                    gm_sb = kp.tile([128, 2048], f32, tag="gmsb")
                    nc.sync.dma_start(gm_sb[:], gm_h.ap()[j])
                    t1 = kp.tile([128, 2048], f32, tag="t1")
                    nc.vector.tensor_mul(t1[:], e_sb[:], gm_sb[:])
                    kraw = kp.tile([128, 128], f32, tag="kraw")
                    nc.vector.tensor_reduce(
                        kraw[:],
                        t1[:].rearrange("p (r g) -> p r g", g=16),
                        mybir.AxisListType.X,
                        ALU.add,
                    )
                    kj = kp.tile([128, 128], f32, tag="kj")
                    nc.vector.tensor_mul(kj[:], kraw[:], r_sb[:])
                    pk = kps.tile([128, 128], f32)
                    nc.tensor.transpose(pk[:], kj[:], ident[:])
                    nc.scalar.copy(kg[j][:], pk[:])

            # ---- gather + multilinear + transpose-back + store ----
            with (
                tc.tile_pool(name="p2", bufs=2) as p2,
                tc.tile_pool(name="p2o", bufs=2) as p2o,
                tc.tile_pool(name="p2ps", bufs=4, space="PSUM") as p2ps,
            ):
                SC = YC // 16  # idx columns per chunk (swdge mode)
                for t in range(NCHUNK):
                    A = p2.tile([128, G, 512], f32, tag="A")
                    Bt = p2.tile([128, G, 512], f32, tag="B")
                    if GATHER_MODE == "indirect":
                        nc.gpsimd.indirect_dma_start(
                            out=A[:],
                            out_offset=None,
                            in_=xT_h.ap()[:, :],
                            in_offset=bass.IndirectOffsetOnAxis(
                                ap=ia_sb[:, t * G : (t + 1) * G], axis=0
                            ),
                        )
                        nc.gpsimd.indirect_dma_start(
                            out=Bt[:],
                            out_offset=None,
                            in_=xT_h.ap()[:, :],
                            in_offset=bass.IndirectOffsetOnAxis(
                                ap=ib_sb[:, t * G : (t + 1) * G], axis=0
                            ),
                        )
                    else:
                        nc.gpsimd.dma_gather(
                            out_ap=A[:],
                            in_ap=xT_h.ap()[:, :],
                            idxs_ap=ia_sb[:, t * SC : (t + 1) * SC],
                            num_idxs=YC,
                            num_idxs_reg=YC,
                            elem_size=B,
                        )
                        nc.gpsimd.dma_gather(
                            out_ap=Bt[:],
                            in_ap=xT_h.ap()[:, :],
                            idxs_ap=ib_sb[:, t * SC : (t + 1) * SC],
                            num_idxs=YC,
                            num_idxs_reg=YC,
                            elem_size=B,
                        )
                    u = p2.tile([128, G, 512], f32, tag="u")
                    v = p2.tile([128, G, 512], f32, tag="v")
                    for gl in range(G):
                        gi = t * G + gl
                        nc.scalar.activation(
                            u[:, gl, :],
                            A[:, gl, :],
                            AF.Identity,
                            bias=kg[0][:, gi : gi + 1],
                            scale=kg[1][:, gi : gi + 1],
                        )
                        nc.scalar.activation(
                            v[:, gl, :],
                            A[:, gl, :],
                            AF.Identity,
                            bias=kg[2][:, gi : gi + 1],
                            scale=kg[3][:, gi : gi + 1],
                        )
                    nc.vector.tensor_mul(v[:], v[:], Bt[:])
                    # out = transpose(u) + transpose(v*B), accumulated in PSUM
                    for nb in range(NB):
                        osb = p2o.tile([128, YC], f32, tag=f"osb{nb}", name=f"osb{nb}")
                        for gh in range(G // 4):
                            ps = p2ps.tile([128, 512], f32)
                            for gq in range(4):
                                gl = gh * 4 + gq
                                nc.tensor.matmul(
                                    ps[:, gq * 128 : (gq + 1) * 128],
                                    u[:, gl, nb * 128 : (nb + 1) * 128],
                                    ident[:],
                                    is_transpose=True,
                                    start=True,
                                    stop=False,
                                )
                                nc.tensor.matmul(
                                    ps[:, gq * 128 : (gq + 1) * 128],
                                    v[:, gl, nb * 128 : (nb + 1) * 128],
                                    ident[:],
                                    is_transpose=True,
                                    start=False,
                                    stop=True,
                                )
                            nc.vector.tensor_copy(
                                osb[:, gh * 512 : (gh + 1) * 512], ps[:]
                            )
                        nc.sync.dma_start(
                            out_h.ap()[
                                nb * 128 : (nb + 1) * 128, t * YC : (t + 1) * YC
                            ],
                            osb[:],
                        )

    nc.compile()
    return nc


def _host_inputs(x, weights, a_idx, b_idx):
    if GATHER_MODE == "indirect":
        ia = _wrap_idx_indirect(np.asarray(a_idx))
        ib = _wrap_idx_indirect(np.asarray(b_idx))
    else:
        ia = _wrap_idx_swdge(np.asarray(a_idx))
        ib = _wrap_idx_swdge(np.asarray(b_idx))
    gm = np.ascontiguousarray(
        np.broadcast_to(
            np.tile(_GATE_COEFFS.T, (1, 128))[:, None, :], (4, 128, 2048)
        )
    ).astype(np.float32)
    ident = np.eye(128, dtype=np.float32)
    weights = np.ascontiguousarray(np.asarray(weights, dtype=np.float32))
    x = np.asarray(x, dtype=np.float32)
    in_maps = []
    for c in range(NCORES):
        in_maps.append(
            {
                "xT": np.ascontiguousarray(x[c * B : (c + 1) * B].T),
                "w16": weights,
                "ia": ia,
                "ib": ib,
                "gm": gm,
                "ident": ident,
            }
        )
    return in_maps


def kernel(x, weights, a_idx, b_idx):
    from concourse.bass_utils import run_bass_kernel_spmd

    if "nc" not in _PROGRAM_CACHE:
        _PROGRAM_CACHE["nc"] = _build_program()
    nc = _PROGRAM_CACHE["nc"]

    in_maps = _host_inputs(x, weights, a_idx, b_idx)
    res = run_bass_kernel_spmd(nc, in_maps, list(range(NCORES)))
    out = np.concatenate([res.results[c]["out"] for c in range(NCORES)], axis=0)
    return out
